# revision 1
# baseline (speedup 1.0000x reference)
# Trainium2 Bass kernel for a cross-attention transformer block.
#
# Reference computation (per batch element b):
#   cond   = conv_w * wrap_pad(cond_emb) + conv_b          [S=128, 256]
#   q      = x @ Wq.T + bq                                 [T, C]
#   k      = cond @ Wk.T + bk                              [S, C]
#   v      = cond @ Wv.T + bv                              [S, C]
#   attn   = softmax(q @ k.T / sqrt(C))                    [T, S]
#   query  = LN1(q + attn @ v)                             [T, C]
#   ff     = gelu(query @ W1.T + b1) @ W2.T + b2           [T, C]
#   out    = LN2(query + ff) + x                           [T, C]
#
# Distribution: pure data-parallel over batch B=32 across 8 NeuronCores
# (4 batch elements per core), one SPMD NEFF.
#
# On-device dataflow keeps activations mostly in feature-major layout
# [feature-on-partition, token-free] so every big matmul runs with free
# dim 512 (full-rate fp32r), with PE-transposes only at the two
# layout-conversion points (z -> z_nat, query_nat -> queryT).

import numpy as np

B, T, C = 32, 1024, 512
NCORES = 8
BPC = B // NCORES          # batch elements per core
L = 252
CC = 256                   # cond channels (L + 4)
S = 128                    # kv sequence length
D = 2 * C                  # FFN hidden
EPS = 1e-5
P = 128
TG = 512                   # token group size
NG = T // TG               # token groups per batch element
NSUB = TG // P             # 128-token subtiles per group
ECH = C // P               # chunks of the C contraction dim
CCH = C // P               # chunks of C on partitions
DCH = D // P               # chunks of D on partitions
JCH = CC // P              # chunks of cond-channel dim
SCALE = 1.0 / np.sqrt(np.float32(C))

_CACHE = {}


def _build(skip_zero_bias=False, identity_ln=False):
    import concourse.bass as bass
    import concourse.tile as tile
    from concourse import bacc, mybir
    from concourse.masks import make_identity

    F32 = mybir.dt.float32
    F32R = mybir.dt.float32r
    BF16 = mybir.dt.bfloat16
    AF = mybir.ActivationFunctionType
    OP = mybir.AluOpType

    nc = bacc.Bacc(
        "TRN2",
        target_bir_lowering=False,
        debug=False,
        enable_asserts=False,
        num_devices=NCORES,
    )

    xT = nc.dram_tensor("xT", [BPC, C, T], F32R, kind="ExternalInput").ap()
    x = nc.dram_tensor("x", [BPC, T, C], F32, kind="ExternalInput").ap()
    padrow = nc.dram_tensor("padrow", [BPC, CC], F32R, kind="ExternalInput").ap()
    WqT = nc.dram_tensor("WqT", [C, C], F32R, kind="ExternalInput").ap()
    WkT = nc.dram_tensor("WkT", [CC, C], F32R, kind="ExternalInput").ap()
    WvT = nc.dram_tensor("WvT", [CC, C], F32R, kind="ExternalInput").ap()
    W1T = nc.dram_tensor("W1T", [C, D], BF16, kind="ExternalInput").ap()
    W2T = nc.dram_tensor("W2T", [D, C], BF16, kind="ExternalInput").ap()
    bq = nc.dram_tensor("bq", [C], F32R, kind="ExternalInput").ap()
    bk = nc.dram_tensor("bk", [C], F32, kind="ExternalInput").ap()
    bv = nc.dram_tensor("bv", [C], F32R, kind="ExternalInput").ap()
    b1 = nc.dram_tensor("b1", [D], F32, kind="ExternalInput").ap()
    b2 = nc.dram_tensor("b2", [C], F32R, kind="ExternalInput").ap()
    conv_w = nc.dram_tensor("conv_w", [S], F32R, kind="ExternalInput").ap()
    conv_b = nc.dram_tensor("conv_b", [S], F32R, kind="ExternalInput").ap()
    ln1_w = nc.dram_tensor("ln1_w", [C], F32, kind="ExternalInput").ap()
    ln1_b = nc.dram_tensor("ln1_b", [C], F32, kind="ExternalInput").ap()
    ln2_w = nc.dram_tensor("ln2_w", [C], F32, kind="ExternalInput").ap()
    ln2_b = nc.dram_tensor("ln2_b", [C], F32, kind="ExternalInput").ap()
    ones_d = nc.dram_tensor("ones_d", [TG], F32R, kind="ExternalInput").ap()
    out = nc.dram_tensor("out", [BPC, T, C], F32, kind="ExternalOutput").ap()

    def bcast_ap(src, parts=P):
        return bass.AP(tensor=src.tensor, offset=src.offset, ap=[[0, parts]] + list(src.ap))

    with tile.TileContext(nc) as tc:
        with (
            tc.tile_pool(name="singles", bufs=1) as singles,
            tc.tile_pool(name="perb", bufs=2) as perb,
            tc.tile_pool(name="xg", bufs=3) as xg,
            tc.tile_pool(name="qt", bufs=1) as qtp,
            tc.tile_pool(name="attn", bufs=2) as attnp,
            tc.tile_pool(name="zt", bufs=1) as ztp,
            tc.tile_pool(name="znat", bufs=1) as znatp,
            tc.tile_pool(name="qnat", bufs=1) as qnatp,
            tc.tile_pool(name="qtt", bufs=1) as qttp,
            tc.tile_pool(name="ht", bufs=1) as htp,
            tc.tile_pool(name="ff", bufs=3) as ffp,
            tc.tile_pool(name="stats", bufs=6) as statsp,
            tc.tile_pool(name="outp", bufs=4) as outp,
            tc.tile_pool(name="psz", bufs=1, space="PSUM") as psz,
            tc.tile_pool(name="psw", bufs=3, space="PSUM") as psw,
            tc.tile_pool(name="psl", bufs=1, space="PSUM") as psl,
        ):
            # ---------- one-time setup ----------
            ident = singles.tile([P, P], F32)
            make_identity(nc, ident)


            if not skip_zero_bias:
                bq_row = singles.tile([1, C], F32R)
                nc.sync.dma_start(bq_row[:], bq[None, :])
                bv_row = singles.tile([1, C], F32R)
                nc.sync.dma_start(bv_row[:], bv[None, :])
                b2_row = singles.tile([1, C], F32R)
                nc.sync.dma_start(b2_row[:], b2[None, :])
                with nc.allow_non_contiguous_dma(reason="small one-time column loads"):
                    bk_col = singles.tile([P, CCH], F32)
                    nc.gpsimd.dma_start(bk_col[:], bk.rearrange("(c p) -> p c", p=P))
                    b1_col = singles.tile([P, DCH], F32)
                    nc.gpsimd.dma_start(b1_col[:], b1.rearrange("(d p) -> p d", p=P))

            if not identity_ln:
                ln1w_bc = singles.tile([P, C], F32)
                nc.gpsimd.dma_start(ln1w_bc[:], bcast_ap(ln1_w))
                ln1b_bc = singles.tile([P, C], F32)
                nc.gpsimd.dma_start(ln1b_bc[:], bcast_ap(ln1_b))
                ln2w_bc = singles.tile([P, C], F32)
                nc.gpsimd.dma_start(ln2w_bc[:], bcast_ap(ln2_w))
                ln2b_bc = singles.tile([P, C], F32)
                nc.gpsimd.dma_start(ln2b_bc[:], bcast_ap(ln2_b))

            ones_row = singles.tile([1, TG], F32R)
            nc.sync.dma_start(ones_row[:], ones_d[None, :])
            ones_col = singles.tile([P, 1], F32R)
            with nc.allow_non_contiguous_dma(reason="one-time ones column"):
                nc.gpsimd.dma_start(ones_col[:], ones_d[0:P, None])
            eps_col = singles.tile([P, 1], F32)
            nc.vector.memset(eps_col[:], EPS)

            convwb2 = singles.tile([2, S], F32R)
            nc.sync.dma_start(convwb2[0:1, :], conv_w[None, :])
            nc.sync.dma_start(convwb2[1:2, :], conv_b[None, :])

            # pad rows on partition 0 (for the K=2 condT outer-product matmul)
            pad2 = singles.tile([2, BPC, CC], F32R)
            for b in range(BPC):
                nc.sync.dma_start(pad2[0:1, b, :], padrow[b][None, :])
                nc.sync.dma_start(pad2[1:2, b, :], ones_d[None, 0:CC])

            # weights after the small latency-critical loads; first-needed first
            wk_t = singles.tile([P, JCH, C], F32R)
            nc.sync.dma_start(wk_t[:], WkT.rearrange("(j p) c -> p j c", p=P))
            wv_t = singles.tile([P, JCH, C], F32R)
            nc.scalar.dma_start(wv_t[:], WvT.rearrange("(j p) c -> p j c", p=P))
            wq_t = singles.tile([P, ECH, C], F32R)
            nc.sync.dma_start(wq_t[:], WqT.rearrange("(e p) c -> p e c", p=P))
            w1_t = singles.tile([P, CCH, D], BF16)
            w1r = W1T.rearrange("(c p) d -> p c d", p=P)
            nc.gpsimd.dma_start(w1_t[:, 0:2, :], w1r[:, 0:2, :])
            nc.scalar.dma_start(w1_t[:, 2:4, :], w1r[:, 2:4, :])
            w2_t = singles.tile([P, DCH, C], BF16)
            w2r = W2T.rearrange("(d p) c -> p d c", p=P)
            nc.gpsimd.dma_start(w2_t[:, 0:4, :], w2r[:, 0:4, :])
            nc.scalar.dma_start(w2_t[:, 4:8, :], w2r[:, 4:8, :])

            def stage_a(b):
                # ---------- stage A: cond / k / v ----------
                # condT[j, s] = pad[j] * conv_w[s] + 1 * conv_b[s]
                condT_ps = psw.tile([P, JCH, S], F32, tag="work")
                for j in range(JCH):
                    nc.tensor.matmul(
                        condT_ps[:, j, :],
                        pad2[:, b, j * P:(j + 1) * P],
                        convwb2[:],
                        start=True, stop=True,
                    )
                condT = perb.tile([P, JCH, S], F32R, tag="condT")
                nc.scalar.copy(condT[:], condT_ps[:])

                # kT[c, s] = sum_j WkT[j, c] condT[j, s] + bk[c]
                kT = perb.tile([P, CCH, S], F32R, tag="kT")
                for ch in range(CCH):
                    kps = psw.tile([P, S], F32, tag="work")
                    for j in range(JCH):
                        nc.tensor.matmul(
                            kps[:],
                            wk_t[:, j, ch * P:(ch + 1) * P],
                            condT[:, j, :],
                            start=(j == 0), stop=(j == JCH - 1),
                        )
                    if skip_zero_bias:
                        nc.scalar.copy(kT[:, ch, :], kps[:])
                    else:
                        nc.scalar.activation(kT[:, ch, :], kps[:], AF.Identity,
                                             bias=bk_col[:, ch:ch + 1])

                # v[s, c] = sum_j condT[j, s] WvT[j, c] + bv[c]
                vps = psw.tile([P, C], F32, tag="work")
                for j in range(JCH):
                    nc.tensor.matmul(
                        vps[:], condT[:, j, :], wv_t[:, j, :],
                        start=(j == 0),
                        stop=(skip_zero_bias and j == JCH - 1),
                        skip_group_check=(j != 0),
                    )
                if not skip_zero_bias:
                    nc.tensor.matmul(
                        vps[:], ones_row[0:1, 0:P], bv_row[:],
                        start=False, stop=True, skip_group_check=True,
                    )
                v_sb = perb.tile([P, C], F32R, tag="v")
                nc.scalar.copy(v_sb[:], vps[:])
                return kT, v_sb

            def qt_block(b, g):
                # ---------- stage B: q projection (feature-major) ----------
                t0 = g * TG
                xT_g = xg.tile([P, ECH, TG], F32R, tag="xT")
                nc.sync.dma_start(
                    xT_g[:],
                    xT[b].rearrange("(e p) t -> p e t", p=P)[:, :, t0:t0 + TG],
                )
                zt_ps = [
                    psz.tile([P, TG], F32, tag=f"zt{ch}", name=f"zt_ps{ch}")
                    for ch in range(CCH)
                ]
                qT = qtp.tile([P, CCH, TG], F32R, tag="qT")
                for ch in range(CCH):
                    for e in range(ECH):
                        nc.tensor.matmul(
                            zt_ps[ch][:],
                            wq_t[:, e, ch * P:(ch + 1) * P],
                            xT_g[:, e, :],
                            start=(e == 0), stop=False,
                            skip_group_check=(e != 0),
                        )
                    if not skip_zero_bias:
                        # + bq[c] (outer product with ones over t)
                        nc.tensor.matmul(
                            zt_ps[ch][:],
                            bq_row[0:1, ch * P:(ch + 1) * P],
                            ones_row[:],
                            start=False, stop=False, skip_group_check=True,
                        )
                    nc.scalar.copy(qT[:, ch, :], zt_ps[ch][:])

                # scores/softmax chain, also one group ahead:
                # scoresT[s, t] = sum_c kT[c, s] qT[c, t]
                kT, v_sb = kv[b]
                sps = psw.tile([P, TG], F32, tag="work")
                for ch in range(CCH):
                    nc.tensor.matmul(
                        sps[:], kT[:, ch, :], qT[:, ch, :],
                        start=(ch == 0), stop=(ch == CCH - 1),
                    )
                pT = attnp.tile([P, TG], F32R, tag="pT")
                nc.scalar.activation(pT[:], sps[:], AF.Exp, scale=float(SCALE))
                lps = psl.tile([1, TG], F32, tag="l")
                nc.tensor.matmul(lps[:], ones_col[:], pT[:],
                                 start=True, stop=True)
                r_sb = attnp.tile([1, TG], F32R, tag="r")
                with nc.allow_low_precision(reason="f32r rounding of softmax sum"):
                    nc.vector.reciprocal(r_sb[:], lps[:])
                rb_ps = psw.tile([P, TG], F32, tag="work")
                nc.tensor.matmul(rb_ps[:], ones_row[0:1, 0:P], r_sb[:],
                                 start=True, stop=True)
                pTn = attnp.tile([P, TG], F32R, tag="pTn")
                nc.vector.tensor_mul(pTn[:], pT[:], rb_ps[:])
                return zt_ps, pTn

            pairs = [(b, g) for b in range(BPC) for g in range(NG)]
            kv = {0: stage_a(0)}
            state = {0: qt_block(*pairs[0])}

            for gi, (b, g) in enumerate(pairs):
                t0 = g * TG
                kT, v_sb = kv[b]
                zt_ps, pTn = state.pop(gi)

                # ---------- stage C: attention ----------
                # attn_outT accumulates into the q PSUM -> zT = q + attn_out
                for ch in range(CCH):
                    nc.tensor.matmul(
                        zt_ps[ch][:],
                        v_sb[:, ch * P:(ch + 1) * P],
                        pTn[:],
                        start=False, stop=True, skip_group_check=True,
                    )
                zT = ztp.tile([P, CCH, TG], F32, tag="zT")
                for ch in range(CCH):
                    nc.vector.tensor_copy(zT[:, ch, :], zt_ps[ch][:])

                # ---------- stage D: z -> token-major, LN1 ----------
                z_nat = znatp.tile([P, NSUB, C], F32, tag="z_nat")
                for sub in range(NSUB):
                    for ch in range(CCH):
                        tps = psw.tile([P, P], F32, tag="work")
                        nc.tensor.transpose(
                            tps[:], zT[:, ch, sub * P:(sub + 1) * P], ident[:]
                        )
                        nc.scalar.copy(z_nat[:, sub, ch * P:(ch + 1) * P], tps[:])

                q_nat = qnatp.tile([P, NSUB, C], F32, tag="q_nat")
                for sub in range(NSUB):
                    st = statsp.tile([P, 6], F32, tag="bn1")
                    nc.vector.bn_stats(st[:], z_nat[:, sub, :])
                    mv = statsp.tile([P, 2], F32, tag="mv1")
                    nc.vector.bn_aggr(mv[:], st[:])
                    rstd = statsp.tile([P, 1], F32, tag="rstd1")
                    nc.scalar.activation(rstd[:], mv[:, 1:2], AF.Sqrt,
                                         bias=eps_col[:])
                    nc.vector.reciprocal(rstd[:], rstd[:])
                    if identity_ln:
                        nc.vector.tensor_scalar(
                            q_nat[:, sub, :], z_nat[:, sub, :],
                            scalar1=mv[:, 0:1], scalar2=rstd[:],
                            op0=OP.subtract, op1=OP.mult,
                        )
                    else:
                        zn = ffp.tile([P, C], F32, tag="zn")
                        nc.vector.tensor_scalar(
                            zn[:], z_nat[:, sub, :],
                            scalar1=mv[:, 0:1], scalar2=rstd[:],
                            op0=OP.subtract, op1=OP.mult,
                        )
                        nc.vector.tensor_mul(zn[:], zn[:], ln1w_bc[:])
                        nc.vector.tensor_add(q_nat[:, sub, :], zn[:], ln1b_bc[:])

                # software pipeline: emit next group's q projection here so the
                # PE fills the LN1-chain stall with next-group matmuls
                if gi + 1 < len(pairs):
                    b2, g2 = pairs[gi + 1]
                    if b2 != b:
                        kv[b2] = stage_a(b2)
                    state[gi + 1] = qt_block(b2, g2)

                # ---------- stage E: query -> feature-major ----------
                qTt = qttp.tile([P, CCH, TG], BF16, tag="qTt")
                for ch in range(CCH):
                    for sub in range(NSUB):
                        tps = psw.tile([P, P], F32, tag="work")
                        nc.tensor.transpose(
                            tps[:], q_nat[:, sub, ch * P:(ch + 1) * P], ident[:]
                        )
                        nc.vector.tensor_copy(
                            qTt[:, ch, sub * P:(sub + 1) * P], tps[:]
                        )

                # ---------- stage F: FFN ----------
                hT = htp.tile([P, DCH, TG], BF16, tag="hT")
                for d in range(DCH):
                    hps = psw.tile([P, TG], F32, tag="work")
                    for ch in range(CCH):
                        nc.tensor.matmul(
                            hps[:],
                            w1_t[:, ch, d * P:(d + 1) * P],
                            qTt[:, ch, :],
                            start=(ch == 0), stop=(ch == CCH - 1),
                        )
                    if skip_zero_bias:
                        nc.scalar.activation(hT[:, d, :], hps[:], AF.Gelu)
                    else:
                        nc.scalar.activation(hT[:, d, :], hps[:], AF.Gelu,
                                             bias=b1_col[:, d:d + 1])

                x_g = xg.tile([P, NSUB, C], F32, tag="x_nat")
                nc.sync.dma_start(
                    x_g[:],
                    x[b][t0:t0 + TG, :].rearrange("(s p) c -> p s c", p=P),
                )

                for sub in range(NSUB):
                    fps = psw.tile([P, C], F32, tag="work")
                    for d in range(DCH):
                        nc.tensor.matmul(
                            fps[:],
                            hT[:, d, sub * P:(sub + 1) * P],
                            w2_t[:, d, :],
                            start=(d == 0),
                            stop=(skip_zero_bias and d == DCH - 1),
                            skip_group_check=(d != 0),
                        )
                    if not skip_zero_bias:
                        nc.tensor.matmul(
                            fps[:], ones_row[0:1, 0:P], b2_row[:],
                            start=False, stop=True, skip_group_check=True,
                        )
                    z2 = ffp.tile([P, C], F32, tag="z2")
                    nc.vector.tensor_add(z2[:], q_nat[:, sub, :], fps[:])

                    st2 = statsp.tile([P, 6], F32, tag="bn2")
                    nc.vector.bn_stats(st2[:], z2[:])
                    mv2 = statsp.tile([P, 2], F32, tag="mv2")
                    nc.vector.bn_aggr(mv2[:], st2[:])
                    rstd2 = statsp.tile([P, 1], F32, tag="rstd2")
                    nc.scalar.activation(rstd2[:], mv2[:, 1:2], AF.Sqrt,
                                         bias=eps_col[:])
                    nc.vector.reciprocal(rstd2[:], rstd2[:])
                    o1 = ffp.tile([P, C], F32, tag="o1")
                    nc.vector.tensor_scalar(
                        o1[:], z2[:],
                        scalar1=mv2[:, 0:1], scalar2=rstd2[:],
                        op0=OP.subtract, op1=OP.mult,
                    )
                    if not identity_ln:
                        nc.vector.tensor_mul(o1[:], o1[:], ln2w_bc[:])
                        nc.vector.tensor_add(o1[:], o1[:], ln2b_bc[:])
                    o2 = outp.tile([P, C], F32, tag="o2")
                    nc.vector.tensor_add(o2[:], o1[:], x_g[:, sub, :])
                    nc.sync.dma_start(
                        out[b][t0 + sub * P:t0 + (sub + 1) * P, :], o2[:]
                    )

    nc.compile()
    return nc


def _get_nc(skip_zero_bias=False, identity_ln=False):
    key = ("nc", skip_zero_bias, identity_ln)
    if key not in _CACHE:
        _CACHE[key] = _build(skip_zero_bias, identity_ln)
    return _CACHE[key]


def _host_prep(inputs):
    x = np.ascontiguousarray(np.asarray(inputs["x"], dtype=np.float32))
    cond_emb = np.asarray(inputs["cond_emb"], dtype=np.float32)
    xT = np.ascontiguousarray(x.transpose(0, 2, 1))
    padrow = np.pad(cond_emb[:, 0, :], ((0, 0), (2, 2)), mode="wrap")
    padrow = np.ascontiguousarray(padrow.astype(np.float32))

    def t(a):
        return np.ascontiguousarray(np.asarray(a, dtype=np.float32).T)

    def c(a):
        return np.ascontiguousarray(np.asarray(a, dtype=np.float32).reshape(-1))

    import ml_dtypes

    shared = {
        "WqT": t(inputs["Wq"]),
        "WkT": t(inputs["Wk"]),
        "WvT": t(inputs["Wv"]),
        "W1T": t(inputs["W1"]).astype(ml_dtypes.bfloat16),
        "W2T": t(inputs["W2"]).astype(ml_dtypes.bfloat16),
        "bq": c(inputs["bq"]),
        "bk": c(inputs["bk"]),
        "bv": c(inputs["bv"]),
        "b1": c(inputs["b1"]),
        "b2": c(inputs["b2"]),
        "conv_w": c(inputs["conv_w"]),
        "conv_b": c(inputs["conv_b"]),
        "ln1_w": c(inputs["ln1_w"]),
        "ln1_b": c(inputs["ln1_b"]),
        "ln2_w": c(inputs["ln2_w"]),
        "ln2_b": c(inputs["ln2_b"]),
    }
    in_maps = []
    for core in range(NCORES):
        lo, hi = core * BPC, (core + 1) * BPC
        m = dict(shared)
        m["x"] = np.ascontiguousarray(x[lo:hi])
        m["xT"] = np.ascontiguousarray(xT[lo:hi])
        m["padrow"] = np.ascontiguousarray(padrow[lo:hi])
        m["ones_d"] = np.ones(TG, np.float32)
        in_maps.append(m)
    return in_maps


def kernel(_trace=False, **inputs):
    from concourse.bass_utils import run_bass_kernel_spmd

    skip_zero_bias = all(
        not np.any(np.asarray(inputs[k])) for k in ("bq", "bk", "bv", "b1", "b2")
    )
    identity_ln = (
        np.all(np.asarray(inputs["ln1_w"]) == 1.0)
        and np.all(np.asarray(inputs["ln2_w"]) == 1.0)
        and not np.any(np.asarray(inputs["ln1_b"]))
        and not np.any(np.asarray(inputs["ln2_b"]))
    )
    nc = _get_nc(skip_zero_bias, identity_ln)
    in_maps = _host_prep(inputs)
    res = run_bass_kernel_spmd(
        nc, in_maps, core_ids=list(range(NCORES)), trace=_trace
    )
    _CACHE["last_results"] = res
    out = np.concatenate([r["out"] for r in res.results], axis=0)
    return np.ascontiguousarray(out.astype(np.float32))

